# revision 8
# baseline (speedup 1.0000x reference)
"""GCN (2x GCNConv + linear + log_softmax) on 8 TRN2 NeuronCores (Bass/Tile).

Layout: core c owns dest nodes [c*nloc,(c+1)*nloc) and all edges into them.
Within a core, gather group g (one Q7 core, 16 partitions) owns edges whose
SOURCE lies in shard g — so the AllGather output [8*16, nloc] (partition
16g+f = feature f of shard g) is the gather table directly, loaded once per
layer.

Host folds the full symmetric norm dinv[src]*ew*dinv[dst] into per-slot
weights (no degree pass on device). Per (dest-node, group) runs are padded
to x4 (light, deg<=16) or x16 (heavy), stride-4/16 DVE reduce produces
partials; a stage-2 ap_gather aligns <=4 partials per node, stride-4 reduce
then a PE matmul with a selector sums across the 8 groups. Edge weights are
replicated across the 16 feature partitions by a tiny PE matmul (sel8) into
PSUM instead of 16 HBM DMAs.
"""
import numpy as np

NCORES = 8
NG = 8          # gather groups = source shards
NQ = 4          # dest-node quarters (bounds partials SBUF)
G = 1536        # slots per gather/mult/reduce chunk (3 PSUM banks fp32)
LIGHT_MAX = 16  # deg <= 16 -> light (stride 4); else heavy (stride 16)


# ---------------------------------------------------------------- host prep
def _seg_cumsum_excl(x2d):
    c = np.cumsum(x2d, axis=1)
    return c - x2d


def _wrap16(a, ncols):
    """[NG, M] -> [16*NG, M//16]; group g element i -> row 16g+(i%16), col i//16."""
    ng, m = a.shape
    out = np.zeros((16 * ng, ncols), dtype=a.dtype)
    j = np.arange(m)
    for g in range(ng):
        out[16 * g + (j % 16), j // 16] = a[g]
    return out


def _prep(src, dst, w, n):
    """Build all per-core device inputs. Returns (statics, list of dicts)."""
    nloc = n // NCORES
    nq = nloc // NQ
    core = dst // nloc
    grp = src // nloc
    sl = src % nloc
    dl = dst % nloc
    q = dl // nq
    nl = dl % nq

    nb = NCORES * NQ * NG * nq
    bucket = ((core * NQ + q) * NG + grp) * nq + nl
    cnt = np.bincount(bucket, minlength=nb)
    assert cnt.max() <= 64, f"node in-degree per (node,group) too high: {cnt.max()}"
    heavy = cnt > LIGHT_MAX
    stride = np.where(heavy, 16, 4)
    slots = ((cnt + stride - 1) // stride) * stride
    lightslots = np.where(heavy, 0, slots)
    heavyslots = np.where(heavy, slots, 0)

    rows = NCORES * NQ * NG
    ls_cs = _seg_cumsum_excl(lightslots.reshape(rows, nq)).reshape(-1)
    hs_cs = _seg_cumsum_excl(heavyslots.reshape(rows, nq)).reshape(-1)
    Lsum = lightslots.reshape(NCORES, NQ, NG, nq).sum(3)
    Hsum = heavyslots.reshape(NCORES, NQ, NG, nq).sum(3)
    r = 64
    Lq = ((Lsum.max(axis=(0, 2)) + r - 1) // r * r).astype(np.int64)
    Hq = (np.maximum((Hsum.max(axis=(0, 2)) + r - 1) // r * r, r)).astype(np.int64)
    Qbase = np.concatenate([[0], np.cumsum(Lq + Hq)])
    NI = int(Qbase[-1])

    b_q = (bucket // (NG * nq)) % NQ  # quarter of each bucket... computed per edge below
    # per-bucket quarter index
    qb_idx = (np.arange(nb) // (NG * nq)) % NQ
    start = np.where(
        heavy,
        Qbase[qb_idx] + Lq[qb_idx] + hs_cs,
        Qbase[qb_idx] + ls_cs,
    )

    order = np.argsort(bucket, kind="stable")
    b_sorted = bucket[order]
    first = np.r_[True, b_sorted[1:] != b_sorted[:-1]]
    run_id = np.cumsum(first) - 1
    first_pos = np.flatnonzero(first)
    pos = np.arange(len(order)) - first_pos[run_id]
    slot = start[b_sorted] + pos

    e_core = core[order]
    e_grp = grp[order]
    flat = (e_core * NG + e_grp) * NI + slot
    idx_all = np.zeros(NCORES * NG * NI, np.int16)
    w_all = np.zeros(NCORES * NG * NI, np.float32)
    idx_all[flat] = sl[order].astype(np.int16)
    w_all[flat] = w[order]
    idx_all = idx_all.reshape(NCORES, NG, NI)
    w_all = w_all.reshape(NCORES, NG, NI)

    # stage-2: <=4 partial positions per (c,q,g,node), 1-based (0 = zero slot)
    npart = (cnt + stride - 1) // stride
    assert npart.max() <= 4
    pstart = np.where(heavy, 1 + Lq[qb_idx] // 4 + hs_cs // 16, 1 + ls_cs // 4)
    ar4 = np.arange(4)
    s2 = np.where(ar4[None, :] < npart[:, None], pstart[:, None] + ar4[None, :], 0)
    # [NC, NQ, NG, nq, 4] -> per (c,g): [NQ, nq*4] padded to s2cols*16
    s2 = s2.reshape(NCORES, NQ, NG, nq * 4).astype(np.int16)
    s2cols = (nq * 4 + 15) // 16
    s2pad = np.zeros((NCORES, NQ, NG, s2cols * 16), np.int16)
    s2pad[:, :, :, : nq * 4] = s2

    Pq = (Lq // 4 + Hq // 16).astype(np.int64)
    Pmax = int(Pq.max())
    assert 1 + Pmax <= 32767

    statics = dict(nloc=nloc, nq=nq, Lq=[int(x) for x in Lq],
                   Hq=[int(x) for x in Hq], Qbase=[int(x) for x in Qbase],
                   NI=NI, s2cols=int(s2cols), Pmax=Pmax)
    per_core = []
    for c in range(NCORES):
        idx_w = _wrap16(idx_all[c], NI // 16)           # [128, NI//16] i16
        s2_w = np.concatenate(
            [_wrap16(s2pad[c, qq], s2cols) for qq in range(NQ)], axis=1
        )                                               # [128, NQ*s2cols] i16
        per_core.append(dict(idx=np.ascontiguousarray(idx_w),
                             s2=np.ascontiguousarray(s2_w),
                             nrm=np.ascontiguousarray(w_all[c])))
    return statics, per_core


# ---------------------------------------------------------------- device
def _build_program(st, in_dim, hid, classes):
    import concourse.bass as bass
    import concourse.bacc as bacc
    import concourse.tile as tile
    from concourse import mybir
    from concourse.masks import make_identity

    nloc, nq = st["nloc"], st["nq"]
    Lq, Hq, Qbase, NI = st["Lq"], st["Hq"], st["Qbase"], st["NI"]
    s2cols, Pmax = st["s2cols"], st["Pmax"]
    S2T = s2cols * 16
    F = hid
    dt = mybir.dt.float32
    import os
    bf = mybir.dt.float32 if os.environ.get("GCN_FP32X") else mybir.dt.bfloat16
    AF = mybir.ActivationFunctionType
    nc = bacc.Bacc("TRN2", target_bir_lowering=False, debug=False,
                   num_devices=NCORES)

    zt_in = nc.dram_tensor("zt", [in_dim, nloc], dt, kind="ExternalInput").ap()
    W1_in = nc.dram_tensor("W1", [in_dim, F], dt, kind="ExternalInput").ap()
    b1_in = nc.dram_tensor("b1", [F, 1], dt, kind="ExternalInput").ap()
    W2_in = nc.dram_tensor("W2", [F, F], bf, kind="ExternalInput").ap()
    b2_in = nc.dram_tensor("b2", [F, 1], dt, kind="ExternalInput").ap()
    Wc_in = nc.dram_tensor("Wc", [F, classes], bf, kind="ExternalInput").ap()
    bc_in = nc.dram_tensor("bc", [classes, 1], dt, kind="ExternalInput").ap()
    sel8_in = nc.dram_tensor("sel8", [NG, 128], dt, kind="ExternalInput").ap()
    sel16_in = nc.dram_tensor("sel16", [128, F], dt, kind="ExternalInput").ap()
    onc_in = nc.dram_tensor("onc", [classes, 1], dt, kind="ExternalInput").ap()
    onr_in = nc.dram_tensor("onr", [1, classes], dt, kind="ExternalInput").ap()
    idx_in = nc.dram_tensor("idx", [128, NI // 16], mybir.dt.int16,
                            kind="ExternalInput").ap()
    s2_in = nc.dram_tensor("s2", [128, NQ * s2cols], mybir.dt.int16,
                           kind="ExternalInput").ap()
    nrm_in = nc.dram_tensor("nrm", [NG, NI], dt, kind="ExternalInput").ap()
    out_ext = nc.dram_tensor("out", [nloc, classes], dt, kind="ExternalOutput").ap()

    h_sh = nc.dram_tensor("h_sh", [F, nloc], dt)
    gt_all = nc.dram_tensor("gt_all", [NCORES * F, nloc], dt, addr_space="Shared")

    # per-quarter chunk lists: (c0_abs, size, stride, partial_base)
    def chunks_of(q):
        ch = []
        for c0 in range(0, Lq[q], G):
            sz = min(G, Lq[q] - c0)
            ch.append((Qbase[q] + c0, sz, 4, 1 + c0 // 4))
        hb = 1 + Lq[q] // 4
        for c0 in range(0, Hq[q], G):
            sz = min(G, Hq[q] - c0)
            ch.append((Qbase[q] + Lq[q] + c0, sz, 16, hb + c0 // 16))
        return ch

    with tile.TileContext(nc) as tc:
        with (
            tc.tile_pool(name="const", bufs=1) as constp,
            tc.tile_pool(name="table", bufs=1) as tablep,
            tc.tile_pool(name="parts", bufs=2) as partsp,
            tc.tile_pool(name="xbuf", bufs=1) as xp_pool,
            tc.tile_pool(name="work", bufs=3) as work,
            tc.tile_pool(name="ewsb", bufs=1) as ewsbp,
            tc.tile_pool(name="idxs", bufs=3) as idxp,
            tc.tile_pool(name="small", bufs=1) as smallp,
            tc.tile_pool(name="psA", bufs=1, space="PSUM") as psA,
            tc.tile_pool(name="psB", bufs=4, space="PSUM") as psB,
        ):
            ident = constp.tile([128, 128], dt, tag="ident")
            make_identity(nc, ident[:])
            sel8 = constp.tile([NG, 128], dt, tag="sel8")
            nc.sync.dma_start(out=sel8[:], in_=sel8_in[:, :])
            sel16 = constp.tile([128, F], dt, tag="sel16")
            nc.sync.dma_start(out=sel16[:], in_=sel16_in[:, :])
            w1t = constp.tile([in_dim, F], dt, tag="w1")
            nc.sync.dma_start(out=w1t[:], in_=W1_in[:, :])
            w2t = constp.tile([F, F], bf, tag="w2")
            nc.sync.dma_start(out=w2t[:], in_=W2_in[:, :])
            wct = constp.tile([F, classes], bf, tag="wc")
            nc.sync.dma_start(out=wct[:], in_=Wc_in[:, :])
            b1t = constp.tile([F, 1], dt, tag="b1")
            nc.sync.dma_start(out=b1t[:], in_=b1_in[:, :])
            b2t = constp.tile([F, 1], dt, tag="b2")
            nc.sync.dma_start(out=b2t[:], in_=b2_in[:, :])
            bct = constp.tile([classes, 1], dt, tag="bc")
            nc.sync.dma_start(out=bct[:], in_=bc_in[:, :])
            onc = constp.tile([classes, 1], dt, tag="onc")
            nc.sync.dma_start(out=onc[:], in_=onc_in[:, :])
            onr = constp.tile([1, classes], dt, tag="onr")
            nc.sync.dma_start(out=onr[:], in_=onr_in[:, :])
            s2t = constp.tile([128, NQ * s2cols], mybir.dt.int16, tag="s2")
            nc.sync.dma_start(out=s2t[:], in_=s2_in[:, :])

            table = tablep.tile([128, nloc], dt, tag="table")
            x_t = xp_pool.tile([F, nloc], bf, tag="x")

            # ---- h1 = W1^T @ z^T -> h_sh
            for j in range(0, nloc, 512):
                m = min(512, nloc - j)
                ztile = work.tile([in_dim, 512], dt, tag="zt", bufs=2)
                nc.sync.dma_start(out=ztile[:, :m], in_=zt_in[:, j:j + m])
                hp = psB.tile([128, 512], dt, tag="mm", space="PSUM")
                nc.tensor.matmul(out=hp[:F, :m], lhsT=w1t[:], rhs=ztile[:, :m],
                                 start=True, stop=True)
                hs = work.tile([F, 512], dt, tag="hs", bufs=2)
                nc.scalar.copy(out=hs[:, :m], in_=hp[:F, :m])
                nc.sync.dma_start(out=h_sh[:, j:j + m], in_=hs[:, :m])

            def allgather():
                nc.gpsimd.collective_compute(
                    "AllGather", mybir.AluOpType.bypass,
                    replica_groups=[list(range(NCORES))],
                    ins=[h_sh[:, :]], outs=[gt_all[:, :]])

            def aggregate(bias_t, relu):
                # table <- gt_all
                qn = nloc // 4
                for j in range(4):
                    nc.sync.dma_start(out=table[:, j * qn:(j + 1) * qn],
                                      in_=gt_all[:, j * qn:(j + 1) * qn])
                for q in range(NQ):
                    parts = partsp.tile([128, 1 + Pmax], dt, tag="parts")
                    nc.vector.memset(parts[:, 0:1], 0.0)
                    for (c0, sz, stride, pbase) in chunks_of(q):
                        idxt = idxp.tile([128, G // 16], mybir.dt.int16, tag="idx")
                        nc.sync.dma_start(
                            out=idxt[:, : sz // 16],
                            in_=idx_in[:, c0 // 16:(c0 + sz) // 16])
                        ewsb = ewsbp.tile([NG, G], dt, tag="ew")
                        nc.sync.dma_start(out=ewsb[:, :sz],
                                          in_=nrm_in[:, c0:c0 + sz])
                        ewps = psA.tile([128, G], dt, tag="ewps", space="PSUM")
                        for k in range(0, sz, 512):
                            m = min(512, sz - k)
                            nc.tensor.matmul(out=ewps[:, k:k + m],
                                             lhsT=sel8[:], rhs=ewsb[:, k:k + m],
                                             start=True, stop=True)
                        msg = work.tile([128, G], dt, tag="msg", bufs=3)
                        nc.gpsimd.ap_gather(
                            out_ap=msg[:, :sz].rearrange("p (n d) -> p n d",
                                                         n=sz, d=1),
                            in_ap=table[:].rearrange("p (n d) -> p n d",
                                                     n=nloc, d=1),
                            idxs_ap=idxt[:, : sz // 16],
                            channels=128, num_elems=nloc, d=1, num_idxs=sz)
                        nc.vector.tensor_tensor(out=msg[:, :sz], in0=msg[:, :sz],
                                                in1=ewps[:, :sz],
                                                op=mybir.AluOpType.mult)
                        nc.vector.tensor_reduce(
                            out=parts[:, pbase:pbase + sz // stride]
                                .rearrange("p (c o) -> p c o", c=sz // stride, o=1),
                            in_=msg[:, :sz].rearrange("p (c s) -> p c s",
                                                      c=sz // stride, s=stride),
                            axis=mybir.AxisListType.X, op=mybir.AluOpType.add)
                    # stage 2 of quarter q
                    for d0 in range(0, S2T, G):
                        sz2 = min(G, S2T - d0)
                        s2g = work.tile([128, G], dt, tag="s2g", bufs=2)
                        nc.gpsimd.ap_gather(
                            out_ap=s2g[:, :sz2].rearrange("p (n d) -> p n d",
                                                          n=sz2, d=1),
                            in_ap=parts[:].rearrange("p (n d) -> p n d",
                                                     n=1 + Pmax, d=1),
                            idxs_ap=s2t[:, q * s2cols + d0 // 16:
                                        q * s2cols + (d0 + sz2) // 16],
                            channels=128, num_elems=1 + Pmax, d=1, num_idxs=sz2)
                        red = work.tile([128, G // 4], dt, tag="red", bufs=2)
                        nc.vector.tensor_reduce(
                            out=red[:, : sz2 // 4].rearrange("p (c o) -> p c o",
                                                             c=sz2 // 4, o=1),
                            in_=s2g[:, :sz2].rearrange("p (c s) -> p c s",
                                                       c=sz2 // 4, s=4),
                            axis=mybir.AxisListType.X, op=mybir.AluOpType.add)
                        n0 = d0 // 4
                        for k in range(0, sz2 // 4, 512):
                            m = min(512, sz2 // 4 - k)
                            m = min(m, nq - (n0 + k))
                            if m <= 0:
                                break
                            xps = psB.tile([128, 512], dt, tag="mm", space="PSUM")
                            nc.tensor.matmul(out=xps[:F, :m], lhsT=sel16[:],
                                             rhs=red[:, k:k + m],
                                             start=True, stop=True)
                            col = q * nq + n0 + k
                            nc.scalar.activation(
                                x_t[:, col:col + m], xps[:F, :m],
                                AF.Relu if relu else AF.Identity,
                                bias=bias_t[:, 0:1], scale=1.0)

            # ---- layer 1
            allgather()
            aggregate(b1t, True)
            # ---- h2 = W2^T @ x1 -> h_sh
            for j in range(0, nloc, 512):
                m = min(512, nloc - j)
                hp2 = psB.tile([128, 512], dt, tag="mm", space="PSUM")
                nc.tensor.matmul(out=hp2[:F, :m], lhsT=w2t[:], rhs=x_t[:, j:j + m],
                                 start=True, stop=True)
                hs2 = work.tile([F, 512], dt, tag="hs", bufs=2)
                nc.scalar.copy(out=hs2[:, :m], in_=hp2[:F, :m])
                nc.sync.dma_start(out=h_sh[:, j:j + m], in_=hs2[:, :m])
            # ---- layer 2
            allgather()
            aggregate(b2t, False)

            # ---- classifier + log_softmax (no max subtraction; |logit| < 2)
            for j in range(0, nloc, 512):
                m = min(512, nloc - j)
                lgp = psB.tile([128, 512], dt, tag="mm", space="PSUM")
                nc.tensor.matmul(out=lgp[:classes, :m], lhsT=wct[:],
                                 rhs=x_t[:, j:j + m], start=True, stop=True)
                lgb = work.tile([classes, 512], dt, tag="lgb", bufs=1)
                nc.scalar.activation(lgb[:, :m], lgp[:classes, :m], AF.Identity,
                                     bias=bct[:, 0:1], scale=1.0)
                ex = work.tile([classes, 512], dt, tag="ex", bufs=1)
                nc.scalar.activation(ex[:, :m], lgb[:, :m], AF.Exp)
                sp = psB.tile([128, 512], dt, tag="mm", space="PSUM")
                nc.tensor.matmul(out=sp[:1, :m], lhsT=onc[:], rhs=ex[:, :m],
                                 start=True, stop=True)
                ls = smallp.tile([1, 512], dt, tag="ls")
                nc.scalar.activation(ls[:, :m], sp[:1, :m], AF.Ln)
                lsb = psB.tile([128, 512], dt, tag="mm", space="PSUM")
                nc.tensor.matmul(out=lsb[:classes, :m], lhsT=onr[:], rhs=ls[:, :m],
                                 start=True, stop=True)
                res = work.tile([classes, 512], dt, tag="res", bufs=1)
                nc.vector.tensor_tensor(out=res[:, :m], in0=lgb[:, :m],
                                        in1=lsb[:classes, :m],
                                        op=mybir.AluOpType.subtract)
                for k in range(0, m, 128):
                    mm = min(128, m - k)
                    tp = psB.tile([128, 512], dt, tag="mm", space="PSUM")
                    nc.tensor.transpose(out=tp[:mm, :classes], in_=res[:, k:k + mm],
                                        identity=ident[:classes, :classes])
                    tp = tp[:, :classes]
                    lt = work.tile([128, classes], dt, tag="lt", bufs=2)
                    nc.vector.tensor_copy(out=lt[:mm, :], in_=tp[:mm, :])
                    nc.sync.dma_start(out=out_ext[j + k:j + k + mm, :],
                                      in_=lt[:mm, :])
    nc.compile()
    return nc


# ---------------------------------------------------------------- entry
def kernel(z, edge_index, edge_attr, W1, b1, W2, b2, Wc, bc, _trace=False):
    from concourse.bass_utils import run_bass_kernel_spmd

    z = np.asarray(z, dtype=np.float32)
    ei = np.asarray(edge_index)
    ea = np.asarray(edge_attr, dtype=np.float32)
    W1 = np.asarray(W1, np.float32); b1 = np.asarray(b1, np.float32)
    W2 = np.asarray(W2, np.float32); b2 = np.asarray(b2, np.float32)
    Wc = np.asarray(Wc, np.float32); bc = np.asarray(bc, np.float32)
    n, in_dim = z.shape
    hid = W1.shape[1]
    classes = Wc.shape[1]
    nloc = n // NCORES

    loops = np.arange(n, dtype=np.int64)
    src = np.concatenate([ei[0].astype(np.int64), loops])
    dst = np.concatenate([ei[1].astype(np.int64), loops])
    ew = np.concatenate([ea, np.ones(n, np.float32)])
    deg = np.zeros(n, np.float32)
    np.add.at(deg, dst, ew)
    dinv = 1.0 / np.sqrt(deg)
    w = (dinv[src] * ew * dinv[dst]).astype(np.float32)

    import os
    _fp32x = bool(os.environ.get("GCN_FP32X"))
    try:
        from ml_dtypes import bfloat16 as _bf16
    except ImportError:
        import jax.numpy as jnp
        _bf16 = jnp.bfloat16
    statics, per_core = _prep(src, dst, w, n)
    nc = _build_program(statics, in_dim, hid, classes)

    F = hid
    sel8 = np.zeros((NG, 128), np.float32)
    for g in range(NG):
        sel8[g, 16 * g:16 * g + F] = 1.0
    sel16 = np.zeros((128, F), np.float32)
    for g in range(NG):
        sel16[16 * g:16 * g + F, :] = np.eye(F, dtype=np.float32)

    in_maps = []
    for c in range(NCORES):
        pc = per_core[c]
        in_maps.append({
            "zt": np.ascontiguousarray(z[c * nloc:(c + 1) * nloc].T),
            "W1": W1, "b1": b1.reshape(F, 1),
            "W2": np.asarray(W2, dtype=(np.float32 if _fp32x else _bf16)),
            "b2": b2.reshape(F, 1),
            "Wc": np.asarray(Wc, dtype=(np.float32 if _fp32x else _bf16)), "bc": bc.reshape(classes, 1),
            "sel8": sel8, "sel16": sel16,
            "onc": np.ones((classes, 1), np.float32),
            "onr": np.ones((1, classes), np.float32),
            "idx": pc["idx"], "s2": pc["s2"], "nrm": pc["nrm"],
        })

    try:
        res = run_bass_kernel_spmd(nc, in_maps, list(range(NCORES)), trace=_trace)
    except ModuleNotFoundError:
        res = run_bass_kernel_spmd(nc, in_maps, list(range(NCORES)))
    out = np.concatenate([res.results[c]["out"] for c in range(NCORES)], axis=0)
    if _trace:
        return out, res
    return out
